# Initial kernel scaffold
#
"""Trainium2 kernel for nn_ConsistencyLoss (batchmean KL vs class-conditional
target distributions).

Reference computation (B = 4,000,000 rows):
    idx    = t if 0 <= t <= 2 else 3            (t in {0,1,2,3} by construction)
    target = normalize(TABLE[idx] + eps)        # [B, 7]
    kl     = sum(target * (log target - log(softmax(x) + eps))) / B

Algebraic decomposition used here (w'_c = normalized table row, ent_c = sum_j
w'_cj ln w'_cj, and sum_j w'_cj = 1):

    kl * B = sum_i logZ_i + sum_c n_c * ent_c - sum_i w'[t_i] . x_i
    sum_i w'[t_i] . x_i = u3 * GX + sum_{c=0..2} delta_c . S_c

where logZ_i = log sum_j e^{x_ij},  GX = sum_{ij} x_ij,
      S_c[j] = sum_{i: t_i = c} x_ij,  n_c = |{i: t_i = c}|,
      u3 = w'_3[0] (row 3 is uniform), delta_c = w'_c - w'_3.

So per row the device only needs a logsumexp plus 3 masked column-sum
groups; the tiny per-class constants are applied on the host in float64.
The log(p + 1e-8) vs log(p) difference is ~1.7e-7 per row on average
(E[1/p] = 1 + 6e for iid normal logits) -> ~3e-7 relative on the result,
far below tolerance, so eps inside the log is ignored.

fatigue_logits is unused by the reference and therefore never touched.

Layout: rows are padded to B' = 8 * 128 * F * NT and sharded data-parallel
across 8 cores. Each 128-partition tile holds F rows per partition stored
column-major ([7, F] per partition) so every per-class-column slice is a
packed fp16 vector -> DVE runs in 2x mode. Inputs are cast to fp16 on the
host (halves DMA; symmetric rounding keeps the mean bias ~1e-6 relative).
Padded rows use x = 0, t = 3: they add exactly ln 7 to logZ and n_pad to
n_3, both corrected analytically on the host.
"""

import sys

import numpy as np

try:
    import concourse.bass as bass  # noqa: F401
except ImportError:
    sys.path.insert(0, "/opt/trn_rl_repo")

import concourse.bass as bass
import concourse.mybir as mybir
from concourse import tile
from concourse.bass_utils import run_bass_kernel_spmd

# ---------------------------------------------------------------- constants
_TABLE = np.array(
    [
        [0.05, 0.02, 0.03, 0.4, 0.05, 0.4, 0.05],
        [0.05, 0.05, 0.05, 0.05, 0.3, 0.05, 0.45],
        [0.1, 0.15, 0.2, 0.02, 0.35, 0.03, 0.15],
        [1.0 / 7.0] * 7,
    ],
    dtype=np.float64,
)
_EPS = 1e-8

B = 4_000_000
NCORES = 8
P = 128
F = 980  # rows per partition per tile (column-major [7, F] blocks)
NT = 4  # tiles per core
R = P * F * NT  # rows per core = 501_760
BP = NCORES * R  # padded batch = 4_014_080

_DT = mybir.dt
_AF = mybir.ActivationFunctionType
_ALU = mybir.AluOpType
_AX = mybir.AxisListType


def build_program(p=P, f=F, nt=NT):
    """One SPMD Bass program; every core runs it on its own row shard.

    Inputs:  xt [nt, p, 7*f] fp16  (x tile, [7, f] per partition)
             tg [nt, p, f]   fp16  (targets, values in {0,1,2,3})
    Outputs: accA [p, nt]      f32  (per-tile  sum_f log(sum_j e^x))
             accB [p, 25*nt]   f32  (per-tile [GX, n0, n1, n2, S_cj x21])
    """
    nc = bass.Bass()
    xt_ext = nc.declare_dram_parameter("xt", [nt, p, 7 * f], _DT.float16, isOutput=False)
    tg_ext = nc.declare_dram_parameter("tg", [nt, p, f], _DT.float16, isOutput=False)
    accA_ext = nc.declare_dram_parameter("accA", [p, nt], _DT.float32, isOutput=True)
    accB_ext = nc.declare_dram_parameter("accB", [p, 25 * nt], _DT.float32, isOutput=True)

    with tile.TileContext(nc) as tc:
        with (
            tc.tile_pool(name="main", bufs=2) as pool,
            tc.tile_pool(name="accp", bufs=1) as accpool,
        ):
            accA = accpool.tile([p, nt], _DT.float32)
            accB = accpool.tile([p, 25 * nt], _DT.float32)

            for ti in range(nt):
                base = 25 * ti
                xt = pool.tile([p, 7 * f], _DT.float16, tag="xt")
                nc.sync.dma_start(out=xt[:], in_=xt_ext[ti])
                tg = pool.tile([p, f], _DT.float16, tag="tg")
                nc.sync.dma_start(out=tg[:], in_=tg_ext[ti])

                # logsumexp: exp on ACT, packed pairwise-add tree on DVE
                e = pool.tile([p, 7 * f], _DT.float16, tag="e")
                nc.scalar.activation(e[:], xt[:], _AF.Exp)

                def col(t_, j):
                    return t_[:, j * f : (j + 1) * f]

                c01 = pool.tile([p, f], _DT.float16, tag="c01")
                nc.vector.tensor_add(c01[:], col(e, 0), col(e, 1))
                c23 = pool.tile([p, f], _DT.float16, tag="c23")
                nc.vector.tensor_add(c23[:], col(e, 2), col(e, 3))
                c45 = pool.tile([p, f], _DT.float16, tag="c45")
                nc.vector.tensor_add(c45[:], col(e, 4), col(e, 5))
                d0 = pool.tile([p, f], _DT.float16, tag="d0")
                nc.vector.tensor_add(d0[:], c01[:], c23[:])
                d1 = pool.tile([p, f], _DT.float16, tag="d1")
                nc.vector.tensor_add(d1[:], c45[:], col(e, 6))
                s32 = pool.tile([p, f], _DT.float32, tag="s32")
                nc.vector.tensor_add(s32[:], d0[:], d1[:])

                lg = pool.tile([p, f], _DT.float32, tag="lg")
                nc.scalar.activation(
                    lg[:], s32[:], _AF.Ln, accum_out=accA[:, ti : ti + 1]
                )

                # grand sum of x (packed fp16 -> 2x reduce)
                nc.vector.tensor_reduce(
                    accB[:, base : base + 1], xt[:], axis=_AX.X, op=_ALU.add
                )

                # per-class masked column sums + counts
                scr = pool.tile([p, f], _DT.float16, tag="scr")
                for c in range(3):
                    m = pool.tile([p, f], _DT.float16, tag=f"m{c}")
                    nc.vector.tensor_scalar(m[:], tg[:], float(c), None, _ALU.is_equal)
                    nc.vector.tensor_reduce(
                        accB[:, base + 1 + c : base + 2 + c],
                        m[:],
                        axis=_AX.X,
                        op=_ALU.add,
                    )
                    for j in range(7):
                        nc.vector.tensor_tensor_reduce(
                            out=scr[:],
                            in0=col(xt, j),
                            in1=m[:],
                            scale=1.0,
                            scalar=0.0,
                            op0=_ALU.mult,
                            op1=_ALU.add,
                            accum_out=accB[
                                :, base + 4 + c * 7 + j : base + 5 + c * 7 + j
                            ],
                        )

            nc.sync.dma_start(out=accA_ext[:], in_=accA[:])
            nc.sync.dma_start(out=accB_ext[:], in_=accB[:])
    return nc


def prep_inputs(emotion_logits, fatigue_targets, p=P, f=F, nt=NT, ncores=NCORES):
    """Pad, cast fp16, shard, and transpose each [p, f, 7] block to [p, 7, f]."""
    b = emotion_logits.shape[0]
    r = p * f * nt
    bp = ncores * r
    xp = np.empty((bp, 7), np.float16)
    xp[:b] = emotion_logits
    xp[b:] = 0.0
    tp = np.empty((bp,), np.float16)
    tp[:b] = fatigue_targets
    tp[b:] = 3.0

    in_maps = []
    for c in range(ncores):
        xc = xp[c * r : (c + 1) * r].reshape(nt, p, f, 7).transpose(0, 1, 3, 2)
        xc = np.ascontiguousarray(xc).reshape(nt, p, 7 * f)
        tc_ = tp[c * r : (c + 1) * r].reshape(nt, p, f)
        in_maps.append({"xt": xc, "tg": tc_})
    return in_maps


def combine(results, b=B, r=R, ncores=NCORES):
    """Host float64 reduction of the per-core accumulators -> scalar KL."""
    w = (_TABLE + _EPS) / (_TABLE + _EPS).sum(axis=1, keepdims=True)
    ent = (w * np.log(w)).sum(axis=1)  # [4]
    u3 = w[3, 0]
    delta = w[:3] - w[3]  # [3, 7]

    n_pad = ncores * r - b
    logz = 0.0
    gx = 0.0
    n = np.zeros(3)
    s = np.zeros((3, 7))
    for res in results:
        acc_a = res["accA"].astype(np.float64)
        acc_b = res["accB"].astype(np.float64).reshape(P, -1, 25)
        logz += acc_a.sum()
        gx += acc_b[:, :, 0].sum()
        n += acc_b[:, :, 1:4].sum(axis=(0, 1))
        s += acc_b[:, :, 4:].sum(axis=(0, 1)).reshape(3, 7)

    logz -= n_pad * np.log(7.0)  # padded rows contribute exactly ln 7 each
    n3 = b - n.sum()
    ent_total = (n * ent[:3]).sum() + n3 * ent[3]
    dot_total = u3 * gx + (delta * s).sum()
    return (logz + ent_total - dot_total) / b


_NC_CACHE = {}


def kernel(fatigue_logits, emotion_logits, fatigue_targets):
    assert emotion_logits.shape == (B, 7)
    if "nc" not in _NC_CACHE:
        _NC_CACHE["nc"] = build_program()
    nc = _NC_CACHE["nc"]
    in_maps = prep_inputs(np.asarray(emotion_logits), np.asarray(fatigue_targets))
    out = run_bass_kernel_spmd(nc, in_maps, list(range(NCORES)))
    kl = combine(out.results)
    return np.float32(kl)


# revision 8
# speedup vs baseline: 1.0018x; 1.0018x over previous
"""Trainium2 kernel for nn_ConsistencyLoss (batchmean KL vs class-conditional
target distributions).

Reference computation (B = 4,000,000 rows):
    idx    = t if 0 <= t <= 2 else 3            (t in {0,1,2,3} by construction)
    target = normalize(TABLE[idx] + eps)        # [B, 7]
    kl     = sum(target * (log target - log(softmax(x) + eps))) / B

Algebraic decomposition used here (w'_c = normalized table row, ent_c = sum_j
w'_cj ln w'_cj, and sum_j w'_cj = 1):

    kl * B = sum_i logZ_i + sum_c n_c * ent_c - sum_i w'[t_i] . x_i
    sum_i w'[t_i] . x_i = u3 * GX + sum_{c=0..2} delta_c . S_c

where logZ_i = log sum_j e^{x_ij},  GX = sum_{ij} x_ij,
      S_c[j] = sum_{i: t_i = c} x_ij,  n_c = |{i: t_i = c}|,
      u3 = w'_3[0] (row 3 is uniform), delta_c = w'_c - w'_3.

So per row the device only needs a logsumexp plus 3 masked column-sum
groups; the tiny per-class constants are applied on the host in float64.
The log(p + 1e-8) vs log(p) difference is ~1.7e-7 per row on average
(E[1/p] = 1 + 6e for iid normal logits) -> ~3e-7 relative on the result,
far below tolerance, so eps inside the log is ignored.

fatigue_logits is unused by the reference and therefore never touched.

Layout: rows are padded to B' = 8 * 128 * F * NT and sharded data-parallel
across 8 cores. Each 128-partition tile holds F rows per partition stored
column-major ([7, F] per partition) so every per-class-column slice is a
packed fp16 vector -> DVE runs in 2x mode. Inputs are cast to fp16 on the
host (halves DMA; symmetric rounding keeps the mean bias ~1e-6 relative).
Padded rows use x = 0, t = 3: they add exactly ln 7 to logZ and n_pad to
n_3, both corrected analytically on the host.
"""

import sys

import numpy as np

try:
    import concourse.bass as bass  # noqa: F401
except ImportError:
    sys.path.insert(0, "/opt/trn_rl_repo")

import concourse.bass as bass
import concourse.mybir as mybir
from concourse import bacc, tile
from concourse.bass_utils import run_bass_kernel_spmd

# ---------------------------------------------------------------- constants
_TABLE = np.array(
    [
        [0.05, 0.02, 0.03, 0.4, 0.05, 0.4, 0.05],
        [0.05, 0.05, 0.05, 0.05, 0.3, 0.05, 0.45],
        [0.1, 0.15, 0.2, 0.02, 0.35, 0.03, 0.15],
        [1.0 / 7.0] * 7,
    ],
    dtype=np.float64,
)
_EPS = 1e-8

B = 4_000_000
NCORES = 8
P = 128
F = 980  # rows per partition per tile (column-major [7, F] blocks)
NT = 4  # tiles per core
R = P * F * NT  # rows per core = 501_760
BP = NCORES * R  # padded batch = 4_014_080

_DT = mybir.dt
_AF = mybir.ActivationFunctionType
_ALU = mybir.AluOpType
_AX = mybir.AxisListType


def build_program(p=P, f=F, nt=NT):
    """One SPMD Bass program; every core runs it on its own row shard.

    Inputs:  xt [nt, p, 7*f] fp16  (x tile, [7, f] per partition)
             tg [nt, p, f]   fp16  (targets, values in {0,1,2,3})
    Outputs: accA [p, nt]      f32  (per-tile  sum_f log(sum_j e^x))
             accB [p, 25*nt]   f32  (per-tile [GX, n0, n1, n2, S_cj x21])
    """
    # Bacc (not raw Bass): its compile() runs generate_event_semaphores
    # (hardware allows at most one sync-wait per instruction) and
    # codegen_inst_isa_subclasses (encodes TensorTensorReduce) — without
    # them walrus fails with "Too many sync wait commands" / "ISA wrong
    # length".
    nc = bacc.Bacc()
    xt_ext = nc.declare_dram_parameter("xt", [nt, p, 7 * f], _DT.float16, isOutput=False)
    tg_ext = nc.declare_dram_parameter("tg", [nt, p, f], _DT.float16, isOutput=False)
    accA_ext = nc.declare_dram_parameter("accA", [p, nt], _DT.float32, isOutput=True)
    accB_ext = nc.declare_dram_parameter("accB", [p, 25 * nt], _DT.float32, isOutput=True)

    with tile.TileContext(nc) as tc:
        with (
            tc.tile_pool(name="main", bufs=2) as pool,
            tc.tile_pool(name="accp", bufs=1) as accpool,
        ):
            accA = accpool.tile([p, nt], _DT.float32)
            accB = accpool.tile([p, 25 * nt], _DT.float32)

            for ti in range(nt):
                base = 25 * ti
                # bufs=nt: input DMAs never reuse a slot, so they carry no
                # WAR sync-waits (DMA instrs have very few wait slots).
                xt = pool.tile([p, 7 * f], _DT.float16, tag="xt", bufs=nt)
                nc.sync.dma_start(out=xt[:], in_=xt_ext[ti])
                tg = pool.tile([p, f], _DT.float16, tag="tg", bufs=nt)
                nc.sync.dma_start(out=tg[:], in_=tg_ext[ti])

                # logsumexp: exp on ACT, packed pairwise-add tree on DVE
                e = pool.tile([p, 7 * f], _DT.float16, tag="e")
                nc.scalar.activation(e[:], xt[:], _AF.Exp)

                def col(t_, j):
                    return t_[:, j * f : (j + 1) * f]

                c01 = pool.tile([p, f], _DT.float16, tag="c01")
                nc.vector.tensor_add(c01[:], col(e, 0), col(e, 1))
                c23 = pool.tile([p, f], _DT.float16, tag="c23")
                nc.vector.tensor_add(c23[:], col(e, 2), col(e, 3))
                c45 = pool.tile([p, f], _DT.float16, tag="c45")
                nc.vector.tensor_add(c45[:], col(e, 4), col(e, 5))
                d0 = pool.tile([p, f], _DT.float16, tag="d0")
                nc.vector.tensor_add(d0[:], c01[:], c23[:])
                d1 = pool.tile([p, f], _DT.float16, tag="d1")
                nc.vector.tensor_add(d1[:], c45[:], col(e, 6))
                s32 = pool.tile([p, f], _DT.float32, tag="s32")
                nc.vector.tensor_add(s32[:], d0[:], d1[:])

                lg = pool.tile([p, f], _DT.float32, tag="lg")
                nc.scalar.activation(lg[:], s32[:], _AF.Ln)
                nc.vector.tensor_reduce(
                    accA[:, ti : ti + 1], lg[:], axis=_AX.X, op=_ALU.add
                )

                # grand sum of x (packed fp16 -> 2x reduce)
                nc.vector.tensor_reduce(
                    accB[:, base : base + 1], xt[:], axis=_AX.X, op=_ALU.add
                )

                # per-class masked column sums + counts
                scr = pool.tile([p, f], _DT.float16, tag="scr")
                for c in range(3):
                    m = pool.tile([p, f], _DT.float16, tag=f"m{c}")
                    nc.vector.tensor_scalar(m[:], tg[:], float(c), None, _ALU.is_equal)
                    nc.vector.tensor_reduce(
                        accB[:, base + 1 + c : base + 2 + c],
                        m[:],
                        axis=_AX.X,
                        op=_ALU.add,
                    )
                    # NOTE: InstTensorTensorReduce compiles + simulates but
                    # hard-crashes the exec unit on HW (NRT status 101), so
                    # the fused mul+reduce is split into two packed-fp16 ops.
                    for j in range(7):
                        nc.vector.tensor_tensor(scr[:], col(xt, j), m[:], _ALU.mult)
                        nc.vector.tensor_reduce(
                            accB[:, base + 4 + c * 7 + j : base + 5 + c * 7 + j],
                            scr[:],
                            axis=_AX.X,
                            op=_ALU.add,
                        )

            nc.sync.dma_start(out=accA_ext[:], in_=accA[:])
            nc.sync.dma_start(out=accB_ext[:], in_=accB[:])
    nc.compile()
    return nc


def prep_inputs(emotion_logits, fatigue_targets, p=P, f=F, nt=NT, ncores=NCORES):
    """Pad, cast fp16, shard, and transpose each [p, f, 7] block to [p, 7, f]."""
    b = emotion_logits.shape[0]
    r = p * f * nt
    bp = ncores * r
    xp = np.empty((bp, 7), np.float16)
    xp[:b] = emotion_logits
    xp[b:] = 0.0
    tp = np.empty((bp,), np.float16)
    tp[:b] = fatigue_targets
    tp[b:] = 3.0

    in_maps = []
    for c in range(ncores):
        xc = xp[c * r : (c + 1) * r].reshape(nt, p, f, 7).transpose(0, 1, 3, 2)
        xc = np.ascontiguousarray(xc).reshape(nt, p, 7 * f)
        tc_ = tp[c * r : (c + 1) * r].reshape(nt, p, f)
        in_maps.append({"xt": xc, "tg": tc_})
    return in_maps


def combine(results, b=B, r=R, ncores=NCORES):
    """Host float64 reduction of the per-core accumulators -> scalar KL."""
    w = (_TABLE + _EPS) / (_TABLE + _EPS).sum(axis=1, keepdims=True)
    ent = (w * np.log(w)).sum(axis=1)  # [4]
    u3 = w[3, 0]
    delta = w[:3] - w[3]  # [3, 7]

    n_pad = ncores * r - b
    logz = 0.0
    gx = 0.0
    n = np.zeros(3)
    s = np.zeros((3, 7))
    for res in results:
        acc_a = res["accA"].astype(np.float64)
        acc_b = res["accB"].astype(np.float64).reshape(P, -1, 25)
        logz += acc_a.sum()
        gx += acc_b[:, :, 0].sum()
        n += acc_b[:, :, 1:4].sum(axis=(0, 1))
        s += acc_b[:, :, 4:].sum(axis=(0, 1)).reshape(3, 7)

    logz -= n_pad * np.log(7.0)  # padded rows contribute exactly ln 7 each
    n3 = b - n.sum()
    ent_total = (n * ent[:3]).sum() + n3 * ent[3]
    dot_total = u3 * gx + (delta * s).sum()
    return (logz + ent_total - dot_total) / b


_NC_CACHE = {}


def kernel(fatigue_logits, emotion_logits, fatigue_targets):
    assert emotion_logits.shape == (B, 7)
    if "nc" not in _NC_CACHE:
        _NC_CACHE["nc"] = build_program()
    nc = _NC_CACHE["nc"]
    in_maps = prep_inputs(np.asarray(emotion_logits), np.asarray(fatigue_targets))
    out = run_bass_kernel_spmd(nc, in_maps, list(range(NCORES)))
    kl = combine(out.results)
    return np.float32(kl)
